# revision 24
# baseline (speedup 1.0000x reference)
"""Trainium2 Bass kernel for nn_CrossAttention (B=2, S=2048, E=1024, H=16, d=64).

Sharding: 8 cores = 2 batches x 4 query-blocks of 512 rows. Each core gets its
query block + the full values[b] for its batch; no collectives needed.

v2 pipeline — three-engine balance (~11us/pair x 8 pairs):
  1. q~T = blk((Wq@Wk@Wv).T) @ qT_in; per-feature bias applied on ACT via a
     Copy-activation (bias rides the per-partition bias port, exp-table-safe).
  2. scores.T = vT_raw-slices @ q~T as K=64 ROW-TILED pairs: head A on PE
     tile (0,0), head B on (64,0) -- the two matmuls run concurrently on
     disjoint 64-row subarrays (measured 2.0x vs serial on HW).
  3. exp split across engines, unit = [128,1024] (one head x 2 kv-tiles):
     - ACT units: exact exp activation (scale=0.125) -> bf16 E.
     - DVE units: Schraudolph magic-add exp: one tensor_scalar
       y = s*(128/ln2*0.125) + (16256 - 7.416 + 1.5*2^23) in f32; the low
       16 bits of each f32 ARE the bf16 bits of e^(s/8) (bit-exact on HW,
       ~3% max approx error; softmax ratio cancels most of it).
       E read back as a stride-2 bf16 view (bitcast) by the PE.
  4. Utilde.T = [v_raw | ones].T @ E per head (M=65 serial; denominator =
     row 64). Wv folded into the output projection on host.
  5. normalize: denom fast-reciprocal on DVE straight from PSUM, partition
     fan-out on GpSimd, outT = U * (1/denom) -> bf16.
  6. out = outT-slices @ wdT' + bias: 4 m-blocks accumulate across 8 PSUM
     banks, bias-add + store overlapping the next block's matmuls.
"""

import sys

for _p in ("/opt/trn_rl_repo",):
    if _p not in sys.path:
        sys.path.insert(0, _p)

from contextlib import ExitStack

import ml_dtypes
import numpy as np

import concourse.bass as bass
import concourse.tile as tile
from concourse import bacc, mybir
from concourse.bass_utils import run_bass_kernel_spmd

F32 = mybir.dt.float32
BF16 = mybir.dt.bfloat16
EXP = mybir.ActivationFunctionType.Exp
IDENT = mybir.ActivationFunctionType.Identity
MULT = mybir.AluOpType.mult
ADD = mybir.AluOpType.add

B, S, E, H, D = 2, 2048, 1024, 16, 64
N_CORES = 8
SQB = S * B // N_CORES  # 512 query rows per core
NP_BF16 = ml_dtypes.bfloat16

# Schraudolph constants for bf16-bits-in-f32-low-halfword exp
SCHRAUD_A = float(np.float32(128.0 / np.log(2.0) * 0.125))
SCHRAUD_C = float(np.float32(16256.0 - 7.416 + 1.5 * 2**23))

# exp routing: unit = both heads of one ktile [128,1024]; listed ktiles run
# the Schraudolph path on DVE, the rest exact exp on ACT
# (balances ACT ~10.4us vs DVE ~10.4us per pair)
DVE_UNITS = {1, 3, 5, 7, 9, 11}

_CACHE = {}


def _build_program():
    nc = bacc.Bacc("TRN2", target_bir_lowering=False, debug=False, num_devices=N_CORES)

    qT_in = nc.dram_tensor("qT_in", [E, SQB], BF16, kind="ExternalInput").ap()
    vT_in = nc.dram_tensor("vT_in", [E, S], BF16, kind="ExternalInput").ap()
    # natural-layout raw values, host-padded to 65 cols/head (col 64 = 1.0)
    vN_in = nc.dram_tensor("vN_in", [S, H * 65], BF16, kind="ExternalInput").ap()
    wsc2 = nc.dram_tensor("wsc2", [128, 128], BF16, kind="ExternalInput").ap()
    csc2 = nc.dram_tensor("csc2", [128, 1], F32, kind="ExternalInput").ap()
    wdT = nc.dram_tensor("wdT", [E, E], BF16, kind="ExternalInput").ap()
    bd_rep = nc.dram_tensor("bd_rep", [128, E], F32, kind="ExternalInput").ap()
    out = nc.dram_tensor("out", [SQB, E], F32, kind="ExternalOutput").ap()

    with tile.TileContext(nc) as tc, ExitStack() as ctx:
        # ---- pools ----
        wpool = ctx.enter_context(tc.tile_pool(name="w", bufs=1))
        outp = ctx.enter_context(tc.tile_pool(name="outp", bufs=1))
        osbp = ctx.enter_context(tc.tile_pool(name="osb", bufs=2))
        # PSUM: 3x[128,1024](6 banks) + 2x[65,512](2 banks) = 8 banks
        sc_ps = ctx.enter_context(tc.tile_pool(name="scps", bufs=3, space="PSUM"))
        u_ps = ctx.enter_context(tc.tile_pool(name="ups", bufs=2, space="PSUM"))
        # per-pair pools live on an inner stack closed before the final
        # output block: their ~60-tile release-semaphore storm then
        # overlaps the last matmuls instead of trailing the program
        inner = ctx.enter_context(ExitStack())
        winner = inner.enter_context(tc.tile_pool(name="winner", bufs=1))
        epa = inner.enter_context(tc.tile_pool(name="epa", bufs=20))  # bf16 E
        epd = inner.enter_context(tc.tile_pool(name="epd", bufs=12))  # f32 E
        vtp = inner.enter_context(tc.tile_pool(name="vtp", bufs=3))
        qtp = inner.enter_context(tc.tile_pool(name="qtp", bufs=2))
        qintp = inner.enter_context(tc.tile_pool(name="qintp", bufs=3))
        rp = inner.enter_context(tc.tile_pool(name="rp", bufs=2))

        # ---- ACT exp-table preload (overlaps the DMA ramp) ----
        dummy = winner.tile([1, 8], F32, tag="dummy")
        nc.gpsimd.memset(dummy[:], 0.0)
        dummy_o = winner.tile([1, 8], BF16, tag="dummy_o")
        nc.scalar.activation(dummy_o[:], dummy[:], EXP)

        # ---- pair-0 critical-path inputs first ----
        qt_in0 = qintp.tile([128, SQB], BF16, tag="qinT", name="qinT0")
        nc.sync.dma_start(qt_in0[:], qT_in[0:128, :])
        wsc2_s = winner.tile([128, 128], BF16, tag="wsc2")
        nc.sync.dma_start(wsc2_s[:], wsc2[:])
        csc2_s = winner.tile([128, 1], F32, tag="csc2")
        nc.sync.dma_start(csc2_s[:], csc2[:])
        vt0a = winner.tile([128, 512], BF16, tag="vt0a")
        nc.sync.dma_start(vt0a[:], vT_in[0:128, 0:512])
        vt0b = winner.tile([128, S - 512], BF16, tag="vt0b")
        nc.sync.dma_start(vt0b[:], vT_in[0:128, 512:S])

        vt_tiles, qt_in_tiles = {}, {}

        def load_pair_inputs(p):
            qt_in = qintp.tile([128, SQB], BF16, tag="qinT", name=f"qinT{p}")
            nc.sync.dma_start(qt_in[:], qT_in[p * 128 : (p + 1) * 128, :])
            vt = vtp.tile([128, S], BF16, tag="vinT", name=f"vinT{p}")
            nc.sync.dma_start(vt[:], vT_in[p * 128 : (p + 1) * 128, :])
            vt_tiles[p], qt_in_tiles[p] = vt, qt_in

        vt_tiles[0], qt_in_tiles[0] = None, qt_in0
        load_pair_inputs(1)
        load_pair_inputs(2)

        def vt_slice(p, rows, t):
            if p == 0:
                if t < 4:
                    return vt0a[rows, t * 128 : (t + 1) * 128]
                return vt0b[rows, (t - 4) * 128 : (t - 3) * 128]
            return vt_tiles[p][rows, t * 128 : (t + 1) * 128]

        # raw values (needed from first U, ~15us in)
        vN = []
        for t in range(16):
            vn = winner.tile([128, H * 65], BF16, tag=f"vN{t}")
            nc.sync.dma_start(vn[:], vN_in[t * 128 : (t + 1) * 128, :])
            vN.append(vn)

        # bulk tail-only weights last
        bd_s = wpool.tile([128, E], F32, tag="bd")
        nc.sync.dma_start(bd_s[:], bd_rep[:])
        wd_s = []
        for kk in range(8):
            t = wpool.tile([128, E], BF16, tag=f"wd{kk}")
            nc.sync.dma_start(t[:], wdT[kk * 128 : (kk + 1) * 128, :])
            wd_s.append(t)

        # E bookkeeping: EAll[p] = list of 16 (tile, is_f32) by (h2, g)
        EAll = {}
        UPS = {}
        outT = []

        def e_slice(p, h2, t):
            """rhs slice [128, 512] for U matmul: head h2 of ktile t."""
            tl, is_f32 = EAll[p][t]
            if not is_f32:
                return tl[:, h2 * 512 : (h2 + 1) * 512]
            bc = tl[:].bitcast(BF16)  # [128, 2048] halfword view
            return bc[:, h2 * 1024 : (h2 + 1) * 1024 : 2]

        def u_mms(p, h2, t, ups):
            h = 2 * p + h2
            nc.tensor.matmul(
                ups[0:65, :], vN[t][:, h * 65 : (h + 1) * 65],
                e_slice(p, h2, t),
                start=(t == 0), stop=(t == 15),
            )

        OT = {}

        def norm_head(p, h2):
            """denom row copy (DVE), fast-reciprocal (DVE), partition
            fan-out (GpSimd), outT half = ups * (1/denom); head A lands
            mid-pair so its U accumulator frees early."""
            ups = UPS[p][h2]
            rd = rp.tile([1, SQB], F32, tag="rd", name=f"rd{2*p+h2}")
            nc.vector.tensor_copy(rd[:], ups[64:65, :])
            rcf = rp.tile([1, SQB], F32, tag="rc", name=f"rc{2*p+h2}")
            nc.vector.reciprocal_approx_fast(rcf[:], rd[:])
            rpsb = rp.tile([64, SQB], F32, tag=f"rpsb{h2}",
                           name=f"rpsb{2*p+h2}")
            nc.gpsimd.partition_broadcast(rpsb[:], rcf[:], channels=64)
            if h2 == 0:
                ot = outp.tile([128, SQB], BF16, tag=f"outT{p}",
                               name=f"outT{p}")
                OT[p] = ot
            else:
                ot = OT[p]
            nc.vector.tensor_mul(ot[h2 * 64 : h2 * 64 + 64, :],
                                 ups[0:64, :], rpsb[:])

        def finish_pair(p):
            UPS.pop(p)
            outT.append(OT.pop(p))

        qt_tiles = {}

        def qproj(p):
            """q~ projection: one matmul + ACT bias-copy (per-partition)."""
            qps = sc_ps.tile([128, 1024], F32, tag="scps")
            nc.tensor.matmul(qps[:, 0:512], wsc2_s[:], qt_in_tiles[p][:],
                             start=True, stop=True)
            qt = qtp.tile([128, SQB], BF16, tag="qT", name=f"qT{p}")
            nc.scalar.activation(qt[:], qps[:, 0:512], IDENT, bias=csc2_s[:])
            qt_tiles[p] = qt

        qproj(0)
        for p in range(9):
            if p > 0:
                upsA = u_ps.tile([128, 512], F32, tag="ups",
                                 name=f"upsA{p-1}")
                upsB = u_ps.tile([128, 512], F32, tag="ups",
                                 name=f"upsB{p-1}")
                UPS[p - 1] = (upsA, upsB)
            if p < 8:
                qt = qt_tiles[p]
                EAll[p] = [None] * 16
                for t in range(16):
                    # one PSUM tile per ktile: cols 0:512 head A (bank i),
                    # 512:1024 head B (bank i+1) -- joint pool event aligns
                    # the ABAB issue; disjoint banks keep the row-tiled
                    # pair concurrency safe
                    pst_ = sc_ps.tile([128, 1024], F32, tag="scps")
                    nc.tensor.matmul(
                        pst_[:, 0:512],
                        vt_slice(p, slice(0, 64), t),
                        qt[0:64, :],
                        start=True, stop=True, tile_position=(0, 0),
                    )
                    nc.tensor.matmul(
                        pst_[:, 512:1024],
                        vt_slice(p, slice(64, 128), t),
                        qt[64:128, :],
                        start=True, stop=True, tile_position=(64, 0),
                    )
                    # exp unit = both heads of ktile t
                    if t in DVE_UNITS:
                        ez = epd.tile([128, 1024], F32, tag="Ez",
                                      name=f"ez{p}_{t}")
                        nc.vector.tensor_scalar(ez[:], pst_[:], SCHRAUD_A,
                                                SCHRAUD_C, MULT, ADD)
                        EAll[p][t] = (ez, True)
                    else:
                        ea = epa.tile([128, 1024], BF16, tag="E",
                                      name=f"ea{p}_{t}")
                        nc.scalar.activation(ea[:], pst_[:], EXP, scale=0.125)
                        EAll[p][t] = (ea, False)
                    # U bursts batched 8-deep at t in {3,7,11,15} to
                    # minimize PE row-mode <-> full-mode switch drains;
                    # qproj rides the first burst (same full mode)
                    if p > 0 and t in (3, 7, 11, 15):
                        uA, uB = UPS[p - 1]
                        h2 = 0 if t <= 7 else 1
                        u0 = {3: 0, 7: 8, 11: 0, 15: 8}[t]
                        for tu in range(u0, u0 + 8):
                            u_mms(p - 1, h2, tu, uA if h2 == 0 else uB)
                    if t == 3 and p + 1 < 8:
                        qproj(p + 1)
                        if p >= 1 and p + 2 < 8:
                            load_pair_inputs(p + 2)
                    if p > 0 and t == 8:
                        norm_head(p - 1, 0)
                    if p > 0 and t == 15:
                        norm_head(p - 1, 1)
                        finish_pair(p - 1)
                        EAll.pop(p - 1)
            else:
                # last pair: U drains behind the final exp units
                uA, uB = UPS[7]
                for t in range(16):
                    u_mms(7, 0, t, uA)
                norm_head(7, 0)
                # output projection k-slices 0..6 for m0-m2 overlap the
                # U(7,B)/norm drain -- keeps the PE dense (and HAM warm)
                # through the tail with useful work
                pst = {}
                for m in (0, 1, 2):
                    pst[m] = sc_ps.tile([128, 1024], F32, tag="scps",
                                        name=f"pst{m}")
                for kk in range(7):
                    for m in (0, 1, 2):
                        for n in range(2):
                            nc.tensor.matmul(
                                pst[m][:, n * 512 : (n + 1) * 512],
                                outT[kk][:, m * 128 : (m + 1) * 128],
                                wd_s[kk][:, n * 512 : (n + 1) * 512],
                                start=(kk == 0), stop=False,
                            )
                for t in range(16):
                    u_mms(7, 1, t, uB)
                norm_head(7, 1)
                finish_pair(7)
                EAll.pop(7)

        # ---- output projection (Wv folded in on host) + bias -> DMA ----
        # all 4 m-blocks accumulate simultaneously: m0-2 on the 3 score
        # tiles, m3 on the two freed U banks
        inner.close()
        m3 = [u_ps.tile([128, 512], F32, tag="ups", name=f"pst3_{n}")
              for n in range(2)]
        for m in range(4):
            if m < 3:
                for n in range(2):
                    nc.tensor.matmul(
                        pst[m][:, n * 512 : (n + 1) * 512],
                        outT[7][:, m * 128 : (m + 1) * 128],
                        wd_s[7][:, n * 512 : (n + 1) * 512],
                        start=False, stop=True,
                    )
            else:
                for kk in range(8):
                    for n in range(2):
                        nc.tensor.matmul(
                            m3[n][:],
                            outT[kk][:, m * 128 : (m + 1) * 128],
                            wd_s[kk][:, n * 512 : (n + 1) * 512],
                            start=(kk == 0), stop=(kk == 7),
                        )
            osb = osbp.tile([128, E], F32, tag="osb")
            if m < 3:
                nc.vector.tensor_add(osb[:], pst[m][:], bd_s[:])
            else:
                nc.vector.tensor_add(osb[:, 0:512], m3[0][:], bd_s[:, 0:512])
                nc.vector.tensor_add(osb[:, 512:1024], m3[1][:],
                                     bd_s[:, 512:1024])
            nc.sync.dma_start(out[m * 128 : (m + 1) * 128, :], osb[:])

    nc.compile()
    return nc


def kernel(queries, values, heads, Wv, bv, Wk, bk, Wq, bq, Wd, bd, **_):
    queries = np.asarray(queries, np.float32)
    values = np.asarray(values, np.float32)
    Wv, bv = np.asarray(Wv, np.float32), np.asarray(bv, np.float32)
    Wk = np.asarray(Wk, np.float32)
    Wq, bq = np.asarray(Wq, np.float32), np.asarray(bq, np.float32)
    Wd, bd = np.asarray(Wd, np.float32), np.asarray(bd, np.float32)
    assert int(heads) == H and queries.shape == (B, S, E)

    if "nc" not in _CACHE:
        _CACHE["nc"] = _build_program()
    nc = _CACHE["nc"]

    def blk(A):
        Z = np.zeros_like(A)
        return np.block([[A, Z], [Z, A]]).astype(NP_BF16)

    Wkv = Wk @ Wv
    # q~ = (Wq@Wkv).T-side fold; k-path bias is softmax-invariant (per-query)
    wsc2 = blk(Wq.T @ Wkv)
    csc2 = np.tile(Wkv.T @ bq, 2)[:, None].astype(np.float32)
    bv_full = np.tile(bv, H)
    bd_rep = np.tile((bd + Wd @ bv_full)[None, :], (128, 1)).astype(np.float32)
    # fold per-head Wv into the dense output projection:
    # out = o_raw @ (Wd @ blkdiag(Wv)).T + (bd + Wd @ tile(bv))
    Wd2 = np.empty_like(Wd)
    for h in range(H):
        Wd2[:, h * D : (h + 1) * D] = Wd[:, h * D : (h + 1) * D] @ Wv
    wdT = np.ascontiguousarray(Wd2.T).astype(NP_BF16)

    vT_b, vN_b = [], []
    for b_ in range(B):
        vT_b.append(np.ascontiguousarray(values[b_].T).astype(NP_BF16))
        v65 = np.ones((S, H, 65), np.float32)
        v65[:, :, 0:64] = values[b_].reshape(S, H, D)
        vN_b.append(np.ascontiguousarray(v65.reshape(S, H * 65)).astype(NP_BF16))

    common = dict(wsc2=wsc2, csc2=csc2, wdT=wdT, bd_rep=bd_rep)
    in_maps = []
    for c in range(N_CORES):
        b_, qb = c // 4, c % 4
        in_maps.append(dict(
            qT_in=np.ascontiguousarray(
                queries[b_, qb * SQB : (qb + 1) * SQB, :].T
            ).astype(NP_BF16),
            vT_in=vT_b[b_],
            vN_in=vN_b[b_],
            **common,
        ))

    _CACHE["last_in_maps"] = in_maps
    res = run_bass_kernel_spmd(nc, in_maps, list(range(N_CORES)))
    out = np.empty((B, S, E), np.float32)
    for c in range(N_CORES):
        b_, qb = c // 4, c % 4
        out[b_, qb * SQB : (qb + 1) * SQB, :] = res.results[c]["out"]
    return out


# revision 25
# speedup vs baseline: 1.1751x; 1.1751x over previous
"""Trainium2 Bass kernel for nn_CrossAttention (B=2, S=2048, E=1024, H=16, d=64).

Sharding: 8 cores = 2 batches x 4 query-blocks of 512 rows. Each core gets its
query block + the full values[b] for its batch; no collectives needed.

v2 pipeline — three-engine balance (~11us/pair x 8 pairs):
  1. q~T = blk((Wq@Wk@Wv).T) @ qT_in; per-feature bias applied on ACT via a
     Copy-activation (bias rides the per-partition bias port, exp-table-safe).
  2. scores.T = vT_raw-slices @ q~T as K=64 ROW-TILED pairs: head A on PE
     tile (0,0), head B on (64,0) -- the two matmuls run concurrently on
     disjoint 64-row subarrays (measured 2.0x vs serial on HW).
  3. exp split across engines, unit = [128,1024] (one head x 2 kv-tiles):
     - ACT units: exact exp activation (scale=0.125) -> bf16 E.
     - DVE units: Schraudolph magic-add exp: one tensor_scalar
       y = s*(128/ln2*0.125) + (16256 - 7.416 + 1.5*2^23) in f32; the low
       16 bits of each f32 ARE the bf16 bits of e^(s/8) (bit-exact on HW,
       ~3% max approx error; softmax ratio cancels most of it).
       E read back as a stride-2 bf16 view (bitcast) by the PE.
  4. Utilde.T = [v_raw | ones].T @ E per head (M=65 serial; denominator =
     row 64). Wv folded into the output projection on host.
  5. normalize: denom fast-reciprocal on DVE straight from PSUM, partition
     fan-out on GpSimd, outT = U * (1/denom) -> bf16.
  6. out = outT-slices @ wdT' + bias: 4 m-blocks accumulate across 8 PSUM
     banks, bias-add + store overlapping the next block's matmuls.
"""

import sys

for _p in ("/opt/trn_rl_repo",):
    if _p not in sys.path:
        sys.path.insert(0, _p)

from contextlib import ExitStack

import ml_dtypes
import numpy as np

import concourse.bass as bass
import concourse.tile as tile
from concourse import bacc, mybir
from concourse.bass_utils import run_bass_kernel_spmd

F32 = mybir.dt.float32
BF16 = mybir.dt.bfloat16
EXP = mybir.ActivationFunctionType.Exp
IDENT = mybir.ActivationFunctionType.Identity
MULT = mybir.AluOpType.mult
ADD = mybir.AluOpType.add

B, S, E, H, D = 2, 2048, 1024, 16, 64
N_CORES = 8
SQB = S * B // N_CORES  # 512 query rows per core
NP_BF16 = ml_dtypes.bfloat16

# Schraudolph constants for bf16-bits-in-f32-low-halfword exp
SCHRAUD_A = float(np.float32(128.0 / np.log(2.0) * 0.125))
SCHRAUD_C = float(np.float32(16256.0 - 7.416 + 1.5 * 2**23))

# exp routing: unit = both heads of one ktile [128,1024]; listed ktiles run
# the Schraudolph path on DVE, the rest exact exp on ACT
# (balances ACT ~10.4us vs DVE ~10.4us per pair)
DVE_UNITS = {1, 3, 5, 7, 9, 11}

_CACHE = {}


def _build_program():
    nc = bacc.Bacc("TRN2", target_bir_lowering=False, debug=False, num_devices=N_CORES)

    qT_in = nc.dram_tensor("qT_in", [E, SQB], BF16, kind="ExternalInput").ap()
    vT_in = nc.dram_tensor("vT_in", [E, S], BF16, kind="ExternalInput").ap()
    # natural-layout raw values, host-padded to 65 cols/head (col 64 = 1.0)
    vN_in = nc.dram_tensor("vN_in", [S, H * 65], BF16, kind="ExternalInput").ap()
    wsc2 = nc.dram_tensor("wsc2", [128, 128], BF16, kind="ExternalInput").ap()
    csc2 = nc.dram_tensor("csc2", [128, 1], F32, kind="ExternalInput").ap()
    wdT = nc.dram_tensor("wdT", [E, E], BF16, kind="ExternalInput").ap()
    bd_rep = nc.dram_tensor("bd_rep", [128, E], F32, kind="ExternalInput").ap()
    out = nc.dram_tensor("out", [SQB, E], F32, kind="ExternalOutput").ap()

    with tile.TileContext(nc) as tc, ExitStack() as ctx:
        # ---- pools ----
        wpool = ctx.enter_context(tc.tile_pool(name="w", bufs=1))
        outp = ctx.enter_context(tc.tile_pool(name="outp", bufs=1))
        osbp = ctx.enter_context(tc.tile_pool(name="osb", bufs=2))
        # PSUM: 3x[128,1024](6 banks) + 2x[65,512](2 banks) = 8 banks
        sc_ps = ctx.enter_context(tc.tile_pool(name="scps", bufs=3, space="PSUM"))
        u_ps = ctx.enter_context(tc.tile_pool(name="ups", bufs=2, space="PSUM"))
        # per-pair pools live on an inner stack closed before the final
        # output block: their ~60-tile release-semaphore storm then
        # overlaps the last matmuls instead of trailing the program
        inner = ctx.enter_context(ExitStack())
        epa = inner.enter_context(tc.tile_pool(name="epa", bufs=20))  # bf16 E
        epd = inner.enter_context(tc.tile_pool(name="epd", bufs=12))  # f32 E
        vtp = inner.enter_context(tc.tile_pool(name="vtp", bufs=3))
        qtp = inner.enter_context(tc.tile_pool(name="qtp", bufs=2))
        qintp = inner.enter_context(tc.tile_pool(name="qintp", bufs=3))
        rp = inner.enter_context(tc.tile_pool(name="rp", bufs=2))

        # ---- ACT exp-table preload (overlaps the DMA ramp) ----
        dummy = wpool.tile([1, 8], F32, tag="dummy")
        nc.gpsimd.memset(dummy[:], 0.0)
        dummy_o = wpool.tile([1, 8], BF16, tag="dummy_o")
        nc.scalar.activation(dummy_o[:], dummy[:], EXP)

        # ---- pair-0 critical-path inputs first ----
        qt_in0 = qintp.tile([128, SQB], BF16, tag="qinT", name="qinT0")
        nc.sync.dma_start(qt_in0[:], qT_in[0:128, :])
        wsc2_s = wpool.tile([128, 128], BF16, tag="wsc2")
        nc.sync.dma_start(wsc2_s[:], wsc2[:])
        csc2_s = wpool.tile([128, 1], F32, tag="csc2")
        nc.sync.dma_start(csc2_s[:], csc2[:])
        vt0a = wpool.tile([128, 512], BF16, tag="vt0a")
        nc.sync.dma_start(vt0a[:], vT_in[0:128, 0:512])
        vt0b = wpool.tile([128, S - 512], BF16, tag="vt0b")
        nc.sync.dma_start(vt0b[:], vT_in[0:128, 512:S])

        vt_tiles, qt_in_tiles = {}, {}

        def load_pair_inputs(p):
            qt_in = qintp.tile([128, SQB], BF16, tag="qinT", name=f"qinT{p}")
            nc.sync.dma_start(qt_in[:], qT_in[p * 128 : (p + 1) * 128, :])
            vt = vtp.tile([128, S], BF16, tag="vinT", name=f"vinT{p}")
            nc.sync.dma_start(vt[:], vT_in[p * 128 : (p + 1) * 128, :])
            vt_tiles[p], qt_in_tiles[p] = vt, qt_in

        vt_tiles[0], qt_in_tiles[0] = None, qt_in0
        load_pair_inputs(1)
        load_pair_inputs(2)

        def vt_slice(p, rows, t):
            if p == 0:
                if t < 4:
                    return vt0a[rows, t * 128 : (t + 1) * 128]
                return vt0b[rows, (t - 4) * 128 : (t - 3) * 128]
            return vt_tiles[p][rows, t * 128 : (t + 1) * 128]

        # raw values (needed from first U, ~15us in)
        vN = []
        for t in range(16):
            vn = wpool.tile([128, H * 65], BF16, tag=f"vN{t}")
            nc.sync.dma_start(vn[:], vN_in[t * 128 : (t + 1) * 128, :])
            vN.append(vn)

        # bulk tail-only weights last
        bd_s = wpool.tile([128, E], F32, tag="bd")
        nc.sync.dma_start(bd_s[:], bd_rep[:])
        wd_s = []
        for kk in range(8):
            t = wpool.tile([128, E], BF16, tag=f"wd{kk}")
            nc.sync.dma_start(t[:], wdT[kk * 128 : (kk + 1) * 128, :])
            wd_s.append(t)

        # E bookkeeping: EAll[p] = list of 16 (tile, is_f32) by (h2, g)
        EAll = {}
        UPS = {}
        outT = []

        def e_slice(p, h2, t):
            """rhs slice [128, 512] for U matmul: head h2 of ktile t."""
            tl, is_f32 = EAll[p][t]
            if not is_f32:
                return tl[:, h2 * 512 : (h2 + 1) * 512]
            bc = tl[:].bitcast(BF16)  # [128, 2048] halfword view
            return bc[:, h2 * 1024 : (h2 + 1) * 1024 : 2]

        def u_mms(p, h2, t, ups):
            h = 2 * p + h2
            nc.tensor.matmul(
                ups[0:65, :], vN[t][:, h * 65 : (h + 1) * 65],
                e_slice(p, h2, t),
                start=(t == 0), stop=(t == 15),
            )

        OT = {}

        def norm_head(p, h2):
            """denom row copy (DVE), fast-reciprocal (DVE), partition
            fan-out (GpSimd), outT half = ups * (1/denom); head A lands
            mid-pair so its U accumulator frees early."""
            ups = UPS[p][h2]
            rd = rp.tile([1, SQB], F32, tag="rd", name=f"rd{2*p+h2}")
            nc.vector.tensor_copy(rd[:], ups[64:65, :])
            rcf = rp.tile([1, SQB], F32, tag="rc", name=f"rc{2*p+h2}")
            nc.vector.reciprocal_approx_fast(rcf[:], rd[:])
            rpsb = rp.tile([64, SQB], F32, tag=f"rpsb{h2}",
                           name=f"rpsb{2*p+h2}")
            nc.gpsimd.partition_broadcast(rpsb[:], rcf[:], channels=64)
            if h2 == 0:
                ot = outp.tile([128, SQB], BF16, tag=f"outT{p}",
                               name=f"outT{p}")
                OT[p] = ot
            else:
                ot = OT[p]
            nc.vector.tensor_mul(ot[h2 * 64 : h2 * 64 + 64, :],
                                 ups[0:64, :], rpsb[:])

        def finish_pair(p):
            UPS.pop(p)
            outT.append(OT.pop(p))

        qt_tiles = {}

        def qproj(p):
            """q~ projection: one matmul + ACT bias-copy (per-partition)."""
            qps = sc_ps.tile([128, 1024], F32, tag="scps")
            nc.tensor.matmul(qps[:, 0:512], wsc2_s[:], qt_in_tiles[p][:],
                             start=True, stop=True)
            qt = qtp.tile([128, SQB], BF16, tag="qT", name=f"qT{p}")
            nc.scalar.activation(qt[:], qps[:, 0:512], IDENT, bias=csc2_s[:])
            qt_tiles[p] = qt

        qproj(0)
        for p in range(9):
            if p > 0:
                upsA = u_ps.tile([128, 512], F32, tag="ups",
                                 name=f"upsA{p-1}")
                upsB = u_ps.tile([128, 512], F32, tag="ups",
                                 name=f"upsB{p-1}")
                UPS[p - 1] = (upsA, upsB)
            if p < 8:
                qt = qt_tiles[p]
                EAll[p] = [None] * 16
                for t in range(16):
                    # one PSUM tile per ktile: cols 0:512 head A (bank i),
                    # 512:1024 head B (bank i+1) -- joint pool event aligns
                    # the ABAB issue; disjoint banks keep the row-tiled
                    # pair concurrency safe
                    pst_ = sc_ps.tile([128, 1024], F32, tag="scps")
                    nc.tensor.matmul(
                        pst_[:, 0:512],
                        vt_slice(p, slice(0, 64), t),
                        qt[0:64, :],
                        start=True, stop=True, tile_position=(0, 0),
                    )
                    nc.tensor.matmul(
                        pst_[:, 512:1024],
                        vt_slice(p, slice(64, 128), t),
                        qt[64:128, :],
                        start=True, stop=True, tile_position=(64, 0),
                    )
                    # exp unit = both heads of ktile t
                    if t in DVE_UNITS:
                        ez = epd.tile([128, 1024], F32, tag="Ez",
                                      name=f"ez{p}_{t}")
                        nc.vector.tensor_scalar(ez[:], pst_[:], SCHRAUD_A,
                                                SCHRAUD_C, MULT, ADD)
                        EAll[p][t] = (ez, True)
                    else:
                        ea = epa.tile([128, 1024], BF16, tag="E",
                                      name=f"ea{p}_{t}")
                        nc.scalar.activation(ea[:], pst_[:], EXP, scale=0.125)
                        EAll[p][t] = (ea, False)
                    # U bursts batched 8-deep at t in {3,7,11,15} to
                    # minimize PE row-mode <-> full-mode switch drains;
                    # qproj rides the first burst (same full mode)
                    if p > 0 and t in (3, 7, 11, 15):
                        uA, uB = UPS[p - 1]
                        h2 = 0 if t <= 7 else 1
                        u0 = {3: 0, 7: 8, 11: 0, 15: 8}[t]
                        for tu in range(u0, u0 + 8):
                            u_mms(p - 1, h2, tu, uA if h2 == 0 else uB)
                    if t == 3 and p + 1 < 8:
                        qproj(p + 1)
                        if p >= 1 and p + 2 < 8:
                            load_pair_inputs(p + 2)
                    if p > 0 and t == 8:
                        norm_head(p - 1, 0)
                    if p > 0 and t == 15:
                        norm_head(p - 1, 1)
                        finish_pair(p - 1)
                        EAll.pop(p - 1)
            else:
                # last pair: U drains behind the final exp units
                uA, uB = UPS[7]
                for t in range(16):
                    u_mms(7, 0, t, uA)
                norm_head(7, 0)
                # output projection k-slices 0..6 for m0-m2 overlap the
                # U(7,B)/norm drain -- keeps the PE dense (and HAM warm)
                # through the tail with useful work
                pst = {}
                for m in (0, 1, 2):
                    pst[m] = sc_ps.tile([128, 1024], F32, tag="scps",
                                        name=f"pst{m}")
                for kk in range(7):
                    for m in (0, 1, 2):
                        for n in range(2):
                            nc.tensor.matmul(
                                pst[m][:, n * 512 : (n + 1) * 512],
                                outT[kk][:, m * 128 : (m + 1) * 128],
                                wd_s[kk][:, n * 512 : (n + 1) * 512],
                                start=(kk == 0), stop=False,
                            )
                for t in range(16):
                    u_mms(7, 1, t, uB)
                norm_head(7, 1)
                finish_pair(7)
                EAll.pop(7)

        # ---- output projection (Wv folded in on host) + bias -> DMA ----
        # all 4 m-blocks accumulate simultaneously: m0-2 on the 3 score
        # tiles, m3 on the two freed U banks
        inner.close()
        m3 = [u_ps.tile([128, 512], F32, tag="ups", name=f"pst3_{n}")
              for n in range(2)]
        for m in range(4):
            if m < 3:
                for n in range(2):
                    nc.tensor.matmul(
                        pst[m][:, n * 512 : (n + 1) * 512],
                        outT[7][:, m * 128 : (m + 1) * 128],
                        wd_s[7][:, n * 512 : (n + 1) * 512],
                        start=False, stop=True,
                    )
            else:
                for kk in range(8):
                    for n in range(2):
                        nc.tensor.matmul(
                            m3[n][:],
                            outT[kk][:, m * 128 : (m + 1) * 128],
                            wd_s[kk][:, n * 512 : (n + 1) * 512],
                            start=(kk == 0), stop=(kk == 7),
                        )
            osb = osbp.tile([128, E], F32, tag="osb")
            if m < 3:
                nc.vector.tensor_add(osb[:], pst[m][:], bd_s[:])
            else:
                nc.vector.tensor_add(osb[:, 0:512], m3[0][:], bd_s[:, 0:512])
                nc.vector.tensor_add(osb[:, 512:1024], m3[1][:],
                                     bd_s[:, 512:1024])
            nc.sync.dma_start(out[m * 128 : (m + 1) * 128, :], osb[:])

    nc.compile()
    return nc


def kernel(queries, values, heads, Wv, bv, Wk, bk, Wq, bq, Wd, bd, **_):
    queries = np.asarray(queries, np.float32)
    values = np.asarray(values, np.float32)
    Wv, bv = np.asarray(Wv, np.float32), np.asarray(bv, np.float32)
    Wk = np.asarray(Wk, np.float32)
    Wq, bq = np.asarray(Wq, np.float32), np.asarray(bq, np.float32)
    Wd, bd = np.asarray(Wd, np.float32), np.asarray(bd, np.float32)
    assert int(heads) == H and queries.shape == (B, S, E)

    if "nc" not in _CACHE:
        _CACHE["nc"] = _build_program()
    nc = _CACHE["nc"]

    def blk(A):
        Z = np.zeros_like(A)
        return np.block([[A, Z], [Z, A]]).astype(NP_BF16)

    Wkv = Wk @ Wv
    # q~ = (Wq@Wkv).T-side fold; k-path bias is softmax-invariant (per-query)
    wsc2 = blk(Wq.T @ Wkv)
    csc2 = np.tile(Wkv.T @ bq, 2)[:, None].astype(np.float32)
    bv_full = np.tile(bv, H)
    bd_rep = np.tile((bd + Wd @ bv_full)[None, :], (128, 1)).astype(np.float32)
    # fold per-head Wv into the dense output projection:
    # out = o_raw @ (Wd @ blkdiag(Wv)).T + (bd + Wd @ tile(bv))
    Wd2 = np.empty_like(Wd)
    for h in range(H):
        Wd2[:, h * D : (h + 1) * D] = Wd[:, h * D : (h + 1) * D] @ Wv
    wdT = np.ascontiguousarray(Wd2.T).astype(NP_BF16)

    vT_b, vN_b = [], []
    for b_ in range(B):
        vT_b.append(np.ascontiguousarray(values[b_].T).astype(NP_BF16))
        v65 = np.ones((S, H, 65), np.float32)
        v65[:, :, 0:64] = values[b_].reshape(S, H, D)
        vN_b.append(np.ascontiguousarray(v65.reshape(S, H * 65)).astype(NP_BF16))

    common = dict(wsc2=wsc2, csc2=csc2, wdT=wdT, bd_rep=bd_rep)
    in_maps = []
    for c in range(N_CORES):
        b_, qb = c // 4, c % 4
        in_maps.append(dict(
            qT_in=np.ascontiguousarray(
                queries[b_, qb * SQB : (qb + 1) * SQB, :].T
            ).astype(NP_BF16),
            vT_in=vT_b[b_],
            vN_in=vN_b[b_],
            **common,
        ))

    _CACHE["last_in_maps"] = in_maps
    res = run_bass_kernel_spmd(nc, in_maps, list(range(N_CORES)))
    out = np.empty((B, S, E), np.float32)
    for c in range(N_CORES):
        b_, qb = c // 4, c % 4
        out[b_, qb * SQB : (qb + 1) * SQB, :] = res.results[c]["out"]
    return out


# revision 26
# speedup vs baseline: 1.1848x; 1.0083x over previous
"""Trainium2 Bass kernel for nn_CrossAttention (B=2, S=2048, E=1024, H=16, d=64).

Sharding: 8 cores = 2 batches x 4 query-blocks of 512 rows. Each core gets its
query block + the full values[b] for its batch; no collectives needed.

v2 pipeline — three-engine balance (~11us/pair x 8 pairs):
  1. q~T = blk((Wq@Wk@Wv).T) @ qT_in; per-feature bias applied on ACT via a
     Copy-activation (bias rides the per-partition bias port, exp-table-safe).
  2. scores.T = vT_raw-slices @ q~T as K=64 ROW-TILED pairs: head A on PE
     tile (0,0), head B on (64,0) -- the two matmuls run concurrently on
     disjoint 64-row subarrays (measured 2.0x vs serial on HW).
  3. exp split across engines, unit = [128,1024] (one head x 2 kv-tiles):
     - ACT units: exact exp activation (scale=0.125) -> bf16 E.
     - DVE units: Schraudolph magic-add exp: one tensor_scalar
       y = s*(128/ln2*0.125) + (16256 - 7.416 + 1.5*2^23) in f32; the low
       16 bits of each f32 ARE the bf16 bits of e^(s/8) (bit-exact on HW,
       ~3% max approx error; softmax ratio cancels most of it).
       E read back as a stride-2 bf16 view (bitcast) by the PE.
  4. Utilde.T = [v_raw | ones].T @ E per head (M=65 serial; denominator =
     row 64). Wv folded into the output projection on host.
  5. normalize: denom fast-reciprocal on DVE straight from PSUM, partition
     fan-out on GpSimd, outT = U * (1/denom) -> bf16.
  6. out = outT-slices @ wdT' + bias: 4 m-blocks accumulate across 8 PSUM
     banks, bias-add + store overlapping the next block's matmuls.
"""

import sys

for _p in ("/opt/trn_rl_repo",):
    if _p not in sys.path:
        sys.path.insert(0, _p)

from contextlib import ExitStack

import ml_dtypes
import numpy as np

import concourse.bass as bass
import concourse.tile as tile
from concourse import bacc, mybir
from concourse.bass_utils import run_bass_kernel_spmd

F32 = mybir.dt.float32
BF16 = mybir.dt.bfloat16
EXP = mybir.ActivationFunctionType.Exp
IDENT = mybir.ActivationFunctionType.Identity
MULT = mybir.AluOpType.mult
ADD = mybir.AluOpType.add

B, S, E, H, D = 2, 2048, 1024, 16, 64
N_CORES = 8
SQB = S * B // N_CORES  # 512 query rows per core
NP_BF16 = ml_dtypes.bfloat16

# Schraudolph constants for bf16-bits-in-f32-low-halfword exp
SCHRAUD_A = float(np.float32(128.0 / np.log(2.0) * 0.125))
SCHRAUD_C = float(np.float32(16256.0 - 7.416 + 1.5 * 2**23))

# exp routing: unit = both heads of one ktile [128,1024]; listed ktiles run
# the Schraudolph path on DVE, the rest exact exp on ACT
# (balances ACT ~10.4us vs DVE ~10.4us per pair)
DVE_UNITS = {1, 3, 5, 7, 9, 11}

_CACHE = {}


def _build_program():
    nc = bacc.Bacc("TRN2", target_bir_lowering=False, debug=False, num_devices=N_CORES)

    qT_in = nc.dram_tensor("qT_in", [E, SQB], BF16, kind="ExternalInput").ap()
    vT_in = nc.dram_tensor("vT_in", [E, S], BF16, kind="ExternalInput").ap()
    # natural-layout raw values, host-padded to 65 cols/head (col 64 = 1.0)
    vN_in = nc.dram_tensor("vN_in", [S, H * 65], BF16, kind="ExternalInput").ap()
    wsc2 = nc.dram_tensor("wsc2", [128, 128], BF16, kind="ExternalInput").ap()
    csc2 = nc.dram_tensor("csc2", [128, 1], F32, kind="ExternalInput").ap()
    wdT = nc.dram_tensor("wdT", [E, E], BF16, kind="ExternalInput").ap()
    bd_rep = nc.dram_tensor("bd_rep", [128, E], F32, kind="ExternalInput").ap()
    out = nc.dram_tensor("out", [SQB, E], F32, kind="ExternalOutput").ap()

    with tile.TileContext(nc) as tc, ExitStack() as ctx:
        # ---- pools ----
        wpool = ctx.enter_context(tc.tile_pool(name="w", bufs=1))
        outp = ctx.enter_context(tc.tile_pool(name="outp", bufs=1))
        osbp = ctx.enter_context(tc.tile_pool(name="osb", bufs=2))
        # PSUM: 3x[128,1024](6 banks) + 2x[65,512](2 banks) = 8 banks
        sc_ps = ctx.enter_context(tc.tile_pool(name="scps", bufs=3, space="PSUM"))
        u_ps = ctx.enter_context(tc.tile_pool(name="ups", bufs=2, space="PSUM"))
        # per-pair pools live on an inner stack closed before the final
        # output block: their ~60-tile release-semaphore storm then
        # overlaps the last matmuls instead of trailing the program
        inner = ctx.enter_context(ExitStack())
        epa = inner.enter_context(tc.tile_pool(name="epa", bufs=20))  # bf16 E
        epd = inner.enter_context(tc.tile_pool(name="epd", bufs=12))  # f32 E
        vtp = inner.enter_context(tc.tile_pool(name="vtp", bufs=3))
        qtp = inner.enter_context(tc.tile_pool(name="qtp", bufs=2))
        qintp = inner.enter_context(tc.tile_pool(name="qintp", bufs=3))
        rp = inner.enter_context(tc.tile_pool(name="rp", bufs=2))

        # ---- ACT exp-table preload (overlaps the DMA ramp) ----
        dummy = wpool.tile([1, 8], F32, tag="dummy")
        nc.gpsimd.memset(dummy[:], 0.0)
        dummy_o = wpool.tile([1, 8], BF16, tag="dummy_o")
        nc.scalar.activation(dummy_o[:], dummy[:], EXP)

        # ---- pair-0 critical-path inputs first ----
        qt_in0 = qintp.tile([128, SQB], BF16, tag="qinT", name="qinT0")
        nc.sync.dma_start(qt_in0[:], qT_in[0:128, :])
        wsc2_s = wpool.tile([128, 128], BF16, tag="wsc2")
        nc.sync.dma_start(wsc2_s[:], wsc2[:])
        csc2_s = wpool.tile([128, 1], F32, tag="csc2")
        nc.sync.dma_start(csc2_s[:], csc2[:])
        vt0a = wpool.tile([128, 512], BF16, tag="vt0a")
        nc.sync.dma_start(vt0a[:], vT_in[0:128, 0:512])
        vt0b = wpool.tile([128, S - 512], BF16, tag="vt0b")
        nc.sync.dma_start(vt0b[:], vT_in[0:128, 512:S])

        vt_tiles, qt_in_tiles = {}, {}

        def load_pair_inputs(p):
            qt_in = qintp.tile([128, SQB], BF16, tag="qinT", name=f"qinT{p}")
            nc.sync.dma_start(qt_in[:], qT_in[p * 128 : (p + 1) * 128, :])
            vt = vtp.tile([128, S], BF16, tag="vinT", name=f"vinT{p}")
            nc.sync.dma_start(vt[:], vT_in[p * 128 : (p + 1) * 128, :])
            vt_tiles[p], qt_in_tiles[p] = vt, qt_in

        vt_tiles[0], qt_in_tiles[0] = None, qt_in0
        load_pair_inputs(1)
        load_pair_inputs(2)

        def vt_slice(p, rows, t):
            if p == 0:
                if t < 4:
                    return vt0a[rows, t * 128 : (t + 1) * 128]
                return vt0b[rows, (t - 4) * 128 : (t - 3) * 128]
            return vt_tiles[p][rows, t * 128 : (t + 1) * 128]

        # raw values (needed from first U, ~15us in)
        vN = []
        for t in range(16):
            vn = wpool.tile([128, H * 65], BF16, tag=f"vN{t}")
            nc.sync.dma_start(vn[:], vN_in[t * 128 : (t + 1) * 128, :])
            vN.append(vn)

        # bulk tail-only weights last
        bd_s = wpool.tile([128, E], F32, tag="bd")
        nc.sync.dma_start(bd_s[:], bd_rep[:])
        wd_s = []
        for kk in range(8):
            t = wpool.tile([128, E], BF16, tag=f"wd{kk}")
            nc.sync.dma_start(t[:], wdT[kk * 128 : (kk + 1) * 128, :])
            wd_s.append(t)

        # E bookkeeping: EAll[p] = list of 16 (tile, is_f32) by (h2, g)
        EAll = {}
        UPS = {}
        outT = []

        def e_slice(p, h2, t):
            """rhs slice [128, 512] for U matmul: head h2 of ktile t."""
            tl, is_f32 = EAll[p][t]
            if not is_f32:
                return tl[:, h2 * 512 : (h2 + 1) * 512]
            bc = tl[:].bitcast(BF16)  # [128, 2048] halfword view
            return bc[:, h2 * 1024 : (h2 + 1) * 1024 : 2]

        def u_mms(p, h2, t, ups):
            h = 2 * p + h2
            nc.tensor.matmul(
                ups[0:65, :], vN[t][:, h * 65 : (h + 1) * 65],
                e_slice(p, h2, t),
                start=(t == 0), stop=(t == 15),
            )

        OT = {}

        def norm_head(p, h2):
            """denom row copy (DVE), fast-reciprocal (DVE), partition
            fan-out (GpSimd), outT half = ups * (1/denom); head A lands
            mid-pair so its U accumulator frees early."""
            ups = UPS[p][h2]
            rd = rp.tile([1, SQB], F32, tag="rd", name=f"rd{2*p+h2}")
            nc.vector.tensor_copy(rd[:], ups[64:65, :])
            rcf = rp.tile([1, SQB], F32, tag="rc", name=f"rc{2*p+h2}")
            nc.vector.reciprocal_approx_fast(rcf[:], rd[:])
            rpsb = rp.tile([64, SQB], F32, tag=f"rpsb{h2}",
                           name=f"rpsb{2*p+h2}")
            nc.gpsimd.partition_broadcast(rpsb[:], rcf[:], channels=64)
            if h2 == 0:
                ot = outp.tile([128, SQB], BF16, tag=f"outT{p}",
                               name=f"outT{p}")
                OT[p] = ot
            else:
                ot = OT[p]
            nc.vector.tensor_mul(ot[h2 * 64 : h2 * 64 + 64, :],
                                 ups[0:64, :], rpsb[:])

        def finish_pair(p):
            UPS.pop(p)
            outT.append(OT.pop(p))

        qt_tiles = {}

        def qproj(p):
            """q~ projection: one matmul + ACT bias-copy (per-partition)."""
            qps = sc_ps.tile([128, 1024], F32, tag="scps")
            nc.tensor.matmul(qps[:, 0:512], wsc2_s[:], qt_in_tiles[p][:],
                             start=True, stop=True)
            qt = qtp.tile([128, SQB], BF16, tag="qT", name=f"qT{p}")
            nc.scalar.activation(qt[:], qps[:, 0:512], IDENT, bias=csc2_s[:])
            qt_tiles[p] = qt

        qproj(0)
        for p in range(9):
            if p > 0:
                upsA = u_ps.tile([128, 512], F32, tag="ups",
                                 name=f"upsA{p-1}")
                upsB = u_ps.tile([128, 512], F32, tag="ups",
                                 name=f"upsB{p-1}")
                UPS[p - 1] = (upsA, upsB)
            if p < 8:
                qt = qt_tiles[p]
                EAll[p] = [None] * 16
                for t in range(16):
                    # one PSUM tile per ktile: cols 0:512 head A (bank i),
                    # 512:1024 head B (bank i+1) -- joint pool event aligns
                    # the ABAB issue; disjoint banks keep the row-tiled
                    # pair concurrency safe
                    pst_ = sc_ps.tile([128, 1024], F32, tag="scps")
                    nc.tensor.matmul(
                        pst_[:, 0:512],
                        vt_slice(p, slice(0, 64), t),
                        qt[0:64, :],
                        start=True, stop=True, tile_position=(0, 0),
                    )
                    nc.tensor.matmul(
                        pst_[:, 512:1024],
                        vt_slice(p, slice(64, 128), t),
                        qt[64:128, :],
                        start=True, stop=True, tile_position=(64, 0),
                    )
                    # exp unit = both heads of ktile t
                    if t in DVE_UNITS:
                        ez = epd.tile([128, 1024], F32, tag="Ez",
                                      name=f"ez{p}_{t}")
                        nc.vector.tensor_scalar(ez[:], pst_[:], SCHRAUD_A,
                                                SCHRAUD_C, MULT, ADD)
                        EAll[p][t] = (ez, True)
                    else:
                        ea = epa.tile([128, 1024], BF16, tag="E",
                                      name=f"ea{p}_{t}")
                        nc.scalar.activation(ea[:], pst_[:], EXP, scale=0.125)
                        EAll[p][t] = (ea, False)
                    # U bursts batched 8-deep at t in {3,7,11,15} to
                    # minimize PE row-mode <-> full-mode switch drains;
                    # qproj rides the first burst (same full mode)
                    if p > 0 and t in (3, 7, 11, 15):
                        uA, uB = UPS[p - 1]
                        h2 = 0 if t <= 7 else 1
                        u0 = {3: 0, 7: 8, 11: 0, 15: 8}[t]
                        for tu in range(u0, u0 + 8):
                            u_mms(p - 1, h2, tu, uA if h2 == 0 else uB)
                    if t == 3 and p + 1 < 8:
                        qproj(p + 1)
                        if p >= 1 and p + 2 < 8:
                            load_pair_inputs(p + 2)
                    if p > 0 and t == 8:
                        norm_head(p - 1, 0)
                    if p > 0 and t == 15:
                        norm_head(p - 1, 1)
                        finish_pair(p - 1)
                        EAll.pop(p - 1)
            else:
                # last pair: U drains behind the final exp units
                uA, uB = UPS[7]
                for t in range(16):
                    u_mms(7, 0, t, uA)
                norm_head(7, 0)
                # output projection k-slices 0..6 for m0-m2 overlap the
                # U(7,B)/norm drain -- keeps the PE dense (and HAM warm)
                # through the tail with useful work
                pst = {}
                for m in (0, 1, 2):
                    pst[m] = sc_ps.tile([128, 1024], F32, tag="scps",
                                        name=f"pst{m}")
                for kk in range(7):
                    for m in (0, 1, 2):
                        for n in range(2):
                            nc.tensor.matmul(
                                pst[m][:, n * 512 : (n + 1) * 512],
                                outT[kk][:, m * 128 : (m + 1) * 128],
                                wd_s[kk][:, n * 512 : (n + 1) * 512],
                                start=(kk == 0), stop=False,
                            )
                for t in range(16):
                    u_mms(7, 1, t, uB)
                norm_head(7, 1)
                finish_pair(7)
                EAll.pop(7)

        # ---- output projection (Wv folded in on host) + bias -> DMA ----
        # all 4 m-blocks accumulate simultaneously: m0-2 on the 3 score
        # tiles, m3 on the two freed U banks
        inner.close()
        m3 = [u_ps.tile([128, 512], F32, tag="ups", name=f"pst3_{n}")
              for n in range(2)]
        for m in range(4):
            if m < 3:
                for n in range(2):
                    nc.tensor.matmul(
                        pst[m][:, n * 512 : (n + 1) * 512],
                        outT[7][:, m * 128 : (m + 1) * 128],
                        wd_s[7][:, n * 512 : (n + 1) * 512],
                        start=False, stop=True,
                    )
            else:
                for kk in range(8):
                    for n in range(2):
                        nc.tensor.matmul(
                            m3[n][:],
                            outT[kk][:, m * 128 : (m + 1) * 128],
                            wd_s[kk][:, n * 512 : (n + 1) * 512],
                            start=(kk == 0), stop=(kk == 7),
                        )
            osb = osbp.tile([128, E], F32, tag="osb")
            if m < 3:
                nc.vector.tensor_add(osb[:], pst[m][:], bd_s[:])
            else:
                nc.vector.tensor_add(osb[:, 0:512], m3[0][:], bd_s[:, 0:512])
                nc.vector.tensor_add(osb[:, 512:1024], m3[1][:],
                                     bd_s[:, 512:1024])
            # ACT-queue dispatch: the SP queue is busy draining the
            # inner pools' release chains at this point
            nc.scalar.dma_start(out[m * 128 : (m + 1) * 128, :], osb[:])

    nc.compile()
    return nc


def kernel(queries, values, heads, Wv, bv, Wk, bk, Wq, bq, Wd, bd, **_):
    queries = np.asarray(queries, np.float32)
    values = np.asarray(values, np.float32)
    Wv, bv = np.asarray(Wv, np.float32), np.asarray(bv, np.float32)
    Wk = np.asarray(Wk, np.float32)
    Wq, bq = np.asarray(Wq, np.float32), np.asarray(bq, np.float32)
    Wd, bd = np.asarray(Wd, np.float32), np.asarray(bd, np.float32)
    assert int(heads) == H and queries.shape == (B, S, E)

    if "nc" not in _CACHE:
        _CACHE["nc"] = _build_program()
    nc = _CACHE["nc"]

    def blk(A):
        Z = np.zeros_like(A)
        return np.block([[A, Z], [Z, A]]).astype(NP_BF16)

    Wkv = Wk @ Wv
    # q~ = (Wq@Wkv).T-side fold; k-path bias is softmax-invariant (per-query)
    wsc2 = blk(Wq.T @ Wkv)
    csc2 = np.tile(Wkv.T @ bq, 2)[:, None].astype(np.float32)
    bv_full = np.tile(bv, H)
    bd_rep = np.tile((bd + Wd @ bv_full)[None, :], (128, 1)).astype(np.float32)
    # fold per-head Wv into the dense output projection:
    # out = o_raw @ (Wd @ blkdiag(Wv)).T + (bd + Wd @ tile(bv))
    Wd2 = np.empty_like(Wd)
    for h in range(H):
        Wd2[:, h * D : (h + 1) * D] = Wd[:, h * D : (h + 1) * D] @ Wv
    wdT = np.ascontiguousarray(Wd2.T).astype(NP_BF16)

    vT_b, vN_b = [], []
    for b_ in range(B):
        vT_b.append(np.ascontiguousarray(values[b_].T).astype(NP_BF16))
        v65 = np.ones((S, H, 65), np.float32)
        v65[:, :, 0:64] = values[b_].reshape(S, H, D)
        vN_b.append(np.ascontiguousarray(v65.reshape(S, H * 65)).astype(NP_BF16))

    common = dict(wsc2=wsc2, csc2=csc2, wdT=wdT, bd_rep=bd_rep)
    in_maps = []
    for c in range(N_CORES):
        b_, qb = c // 4, c % 4
        in_maps.append(dict(
            qT_in=np.ascontiguousarray(
                queries[b_, qb * SQB : (qb + 1) * SQB, :].T
            ).astype(NP_BF16),
            vT_in=vT_b[b_],
            vN_in=vN_b[b_],
            **common,
        ))

    _CACHE["last_in_maps"] = in_maps
    res = run_bass_kernel_spmd(nc, in_maps, list(range(N_CORES)))
    out = np.empty((B, S, E), np.float32)
    for c in range(N_CORES):
        b_, qb = c // 4, c % 4
        out[b_, qb * SQB : (qb + 1) * SQB, :] = res.results[c]["out"]
    return out
